# revision 11
# baseline (speedup 1.0000x reference)
"""MoE layer kernel for 8 Trainium2 NeuronCores.

Strategy (expert-parallel, host routing):
  - Gating network (tiny, <1% of FLOPs) is computed on host with jax-CPU,
    replicating the reference ops exactly -> probs / top-k / combine
    weights / load-balance loss are numerically faithful.
  - Tokens are dispatched by top-k expert index on host ("all-to-all" done
    during input sharding).  Each expert's token list is split into
    fixed-capacity slots; every core gets S slots of capacity C (SPMD
    uniform shapes).  Expert FFN (4 matmul layers + 2 layernorms,
    ~97% of FLOPs, all the big weights) runs on device in bf16 with fp32
    accumulation, feature-major activation layout (no transposes needed).
  - Outputs are combined (scatter-add with top-k weights) on host.
"""

import os
import sys
from contextlib import ExitStack

for _p in ("/opt/trn_rl_repo",):
    if _p not in sys.path and os.path.isdir(_p):
        sys.path.insert(0, _p)

import numpy as np
import ml_dtypes

BF16 = ml_dtypes.bfloat16

E, D, H, O, TOPK = 12, 1024, 2048, 1024, 4
HH, G1, G2 = H // 2, 256, 128
B = 4096
N_CORES = 8
KD, KH, KHH, KO = D // 128, H // 128, HH // 128, O // 128  # 8, 16, 8, 8
CAP_MAX = 768  # max slot capacity (SBUF budget)

last_result = None  # BassKernelResults of the most recent run (for test.py)
_stash = None  # (nc, in_maps) of the most recent run (for bench_exec)


# ----------------------------------------------------------------------------
# Host gating (exact replica of reference gating, on CPU)
# ----------------------------------------------------------------------------

def _gating(inputs):
    import jax
    import jax.numpy as jnp

    cpu = jax.devices("cpu")[0]
    with jax.default_device(cpu):
        x = jnp.asarray(np.asarray(inputs["x"]))
        gw_in = jnp.asarray(np.asarray(inputs["gw_in"]))
        gb_in = jnp.asarray(np.asarray(inputs["gb_in"]))
        gln1_g = jnp.asarray(np.asarray(inputs["gln1_g"]))
        gln1_b = jnp.asarray(np.asarray(inputs["gln1_b"]))
        gw_1 = jnp.asarray(np.asarray(inputs["gw_1"]))
        gb_1 = jnp.asarray(np.asarray(inputs["gb_1"]))
        gln2_g = jnp.asarray(np.asarray(inputs["gln2_g"]))
        gln2_b = jnp.asarray(np.asarray(inputs["gln2_b"]))
        gw_2 = jnp.asarray(np.asarray(inputs["gw_2"]))
        gb_2 = jnp.asarray(np.asarray(inputs["gb_2"]))
        gw_out = jnp.asarray(np.asarray(inputs["gw_out"]))
        gb_out = jnp.asarray(np.asarray(inputs["gb_out"]))
        temperature = jnp.asarray(np.asarray(inputs["temperature"]))

        def _ln(v, g, b):
            m = v.mean(-1, keepdims=True)
            var = ((v - m) ** 2).mean(-1, keepdims=True)
            return (v - m) * jax.lax.rsqrt(var + 1e-5) * g + b

        g0 = jax.nn.relu(x @ gw_in + gb_in)
        g1 = jax.nn.relu(_ln(g0, gln1_g, gln1_b)) @ gw_1 + gb_1 + g0
        g2 = jax.nn.relu(_ln(g1, gln2_g, gln2_b)) @ gw_2 + gb_2
        logits = (g2 @ gw_out + gb_out) / temperature
        probs = jax.nn.softmax(logits, axis=1)
        topp, topi = jax.lax.top_k(probs, TOPK)
        topp = topp / topp.sum(axis=1, keepdims=True)

        usage = probs.mean(axis=0)
        ideal = 1.0 / E
        kl = jnp.sum(ideal * (jnp.log(ideal) - jnp.log(usage + 1e-8)))
        var = jnp.sum((usage - ideal) ** 2)
        ent = -jnp.sum(usage * jnp.log(usage + 1e-8))
        ent_loss = 1.0 - ent / np.log(E).astype(np.float32)
        loss = (0.5 * kl + 0.3 * var + 0.2 * ent_loss) * 0.05

    return (
        np.asarray(probs, dtype=np.float32),
        np.asarray(topi, dtype=np.int32),
        np.asarray(topp, dtype=np.float32),
        np.asarray(loss, dtype=np.float32),
    )


# ----------------------------------------------------------------------------
# Slot planning: split per-expert token lists into 8*S fixed-capacity slots
# ----------------------------------------------------------------------------

def _plan(topi):
    idx_per_e = []
    for e in range(E):
        rows = np.nonzero((topi == e).any(axis=1))[0].astype(np.int32)
        idx_per_e.append(rows)
    counts = np.array([len(v) for v in idx_per_e])
    assert counts.sum() == B * TOPK

    best = None
    for S in (2, 3, 4, 5, 6):
        for C in range(128, CAP_MAX + 1, 64):
            nslots = int(np.ceil(counts / C).sum())
            if nslots <= N_CORES * S:
                cost = S * C
                if best is None or cost < best[0] or (cost == best[0] and S < best[1]):
                    best = (cost, S, C)
                break  # larger C for same S only costs more
    assert best is not None, f"no feasible slot plan for counts {counts}"
    _, S, C = best

    slots = []  # (expert, idx array)
    for e in range(E):
        v = idx_per_e[e]
        nseg = max(1, int(np.ceil(len(v) / C)))
        for chunk in np.array_split(v, nseg):
            slots.append((e, chunk))
    while len(slots) < N_CORES * S:
        slots.append((0, np.zeros((0,), np.int32)))
    # Largest segments first, round-robin over cores for balance of DMA/compute
    slots.sort(key=lambda t: -len(t[1]))
    per_core = [[] for _ in range(N_CORES)]
    for i, sl in enumerate(slots):
        per_core[i % N_CORES].append(sl)
    return S, C, per_core


# ----------------------------------------------------------------------------
# Device kernel (SPMD program, built once per (S, C))
# ----------------------------------------------------------------------------

_BUILD_CACHE = {}


def _build(S, C):
    key = (S, C)
    if key in _BUILD_CACHE:
        return _BUILD_CACHE[key]

    import concourse.bass as bass
    import concourse.tile as tile
    from concourse import bacc, mybir

    f32 = mybir.dt.float32
    bf16 = mybir.dt.bfloat16
    AF = mybir.ActivationFunctionType

    nc = bacc.Bacc(
        "TRN2",
        target_bir_lowering=False,
        debug=False,
        num_devices=N_CORES,
    )

    xg = nc.dram_tensor("xg", [S, 128, KD, C], bf16, kind="ExternalInput").ap()
    w1 = nc.dram_tensor("w1", [S, KH, 128, KD, 128], bf16, kind="ExternalInput").ap()
    w2 = nc.dram_tensor("w2", [S, KH, 128, KH, 128], bf16, kind="ExternalInput").ap()
    w3 = nc.dram_tensor("w3", [S, KHH, 128, KH, 128], bf16, kind="ExternalInput").ap()
    w4 = nc.dram_tensor("w4", [S, KO, 128, KHH, 128], bf16, kind="ExternalInput").ap()
    # per-feature vectors: [b_in, ln1_g, ln1_b, b_h1, ln2_g, ln2_b] (H-sized)
    vec2 = nc.dram_tensor("vec2", [S, 128, 6, KH], f32, kind="ExternalInput").ap()
    # [b_h2, b_out] (HH/O-sized)
    vec1 = nc.dram_tensor("vec1", [S, 128, 2, KHH], f32, kind="ExternalInput").ap()
    yo = nc.dram_tensor("yo", [S, 128, KO, C], bf16, kind="ExternalOutput").ap()

    nch = [(o, min(512, C - o)) for o in range(0, C, 512)]

    with ExitStack() as ctx:
        tc = ctx.enter_context(tile.TileContext(nc))
        const = ctx.enter_context(tc.tile_pool(name="const", bufs=1))
        ones = const.tile([128, 1], bf16)
        nc.vector.memset(ones, 1.0)
        ones1f = const.tile([1, 128], f32)
        nc.vector.memset(ones1f, 1.0)
        eps1 = const.tile([1, 1], f32)
        nc.vector.memset(eps1, 1e-5)

        wpool = ctx.enter_context(tc.tile_pool(name="wpool", bufs=6))
        apool = ctx.enter_context(tc.tile_pool(name="apool", bufs=1))
        vpool = ctx.enter_context(tc.tile_pool(name="vpool", bufs=2))
        rows = ctx.enter_context(tc.tile_pool(name="rows", bufs=2))
        bpool = ctx.enter_context(tc.tile_pool(name="bpool", bufs=2))
        mpsum = ctx.enter_context(tc.tile_pool(name="mpsum", bufs=2, space="PSUM"))
        spsum = ctx.enter_context(tc.tile_pool(name="spsum", bufs=1, space="PSUM"))
        bpsum = ctx.enter_context(tc.tile_pool(name="bpsum", bufs=1, space="PSUM"))

        def mm_layer(s, wdram, Kt, Mt, rhs, epilogue, wtag):
            for m in range(Mt):
                wt = wpool.tile([128, Kt, 128], bf16, tag="w", name=f"wt_{wtag}")
                nc.sync.dma_start(out=wt, in_=wdram[s, m])
                pss = [
                    mpsum.tile([128, wd], f32, tag=f"mm{i}", name="ps")
                    for i, (o, wd) in enumerate(nch)
                ]
                # k outer, chunk inner: both chunks reuse the freshly loaded
                # k-tile of weights -> one LDWEIGHTS per (m, k), not per mm
                for k in range(Kt):
                    for i, (o, wd) in enumerate(nch):
                        nc.tensor.matmul(
                            pss[i],
                            wt[:, k, :],
                            rhs[:, k, o : o + wd],
                            start=(k == 0),
                            stop=(k == Kt - 1),
                        )
                for i, (o, wd) in enumerate(nch):
                    epilogue(m, o, wd, pss[i])

        def layernorm(Tin, Kt, g_ap, b_ap, act_func, Tout):
            Hdim = Kt * 128
            # per-token sum / sum-of-squares over the partition (feature) dim
            mean = rows.tile([1, C], f32, tag="mean", name="mean")
            rstd = rows.tile([1, C], f32, tag="rstd", name="rstd")
            for i, (o, wd) in enumerate(nch):
                pss = spsum.tile([1, wd], f32, tag="pss", name="pss")
                for k in range(Kt):
                    nc.tensor.matmul(
                        pss,
                        ones,
                        Tin[:, k, o : o + wd],
                        start=(k == 0),
                        stop=(k == Kt - 1),
                    )
                psq = spsum.tile([1, wd], f32, tag="psq", name="psq")
                for k in range(Kt):
                    sqc = bpool.tile([128, 512], bf16, tag="sqc", bufs=3, name="sqc")
                    nc.scalar.square(sqc[:, :wd], Tin[:, k, o : o + wd])
                    nc.tensor.matmul(
                        psq,
                        ones,
                        sqc[:, :wd],
                        start=(k == 0),
                        stop=(k == Kt - 1),
                    )
                nc.scalar.mul(mean[:, o : o + wd], pss, 1.0 / Hdim)
                m2 = rows.tile([1, 512], f32, tag="m2", name="m2")
                nc.scalar.square(m2[:, :wd], mean[:, o : o + wd])
                var = rows.tile([1, 512], f32, tag="var", name="var")
                nc.vector.scalar_tensor_tensor(
                    out=var[:, :wd],
                    in0=psq,
                    scalar=1.0 / Hdim,
                    in1=m2[:, :wd],
                    op0=mybir.AluOpType.mult,
                    op1=mybir.AluOpType.subtract,
                )
                sd = rows.tile([1, 512], f32, tag="sd", name="sd")
                nc.scalar.activation(sd[:, :wd], var[:, :wd], AF.Sqrt, bias=eps1)
                nc.vector.reciprocal(rstd[:, o : o + wd], sd[:, :wd])
            # broadcast mean/rstd across partitions with a K=1 fp32 matmul
            for i, (o, wd) in enumerate(nch):
                Mb = bpsum.tile([128, wd], f32, tag="Mb", name="Mb")
                nc.tensor.matmul(Mb, ones1f, mean[:, o : o + wd], start=True, stop=True)
                Rb = bpsum.tile([128, wd], f32, tag="Rb", name="Rb")
                nc.tensor.matmul(Rb, ones1f, rstd[:, o : o + wd], start=True, stop=True)
                for k in range(Kt):
                    t0 = bpool.tile([128, 512], f32, tag="t0", name="t0")
                    nc.vector.tensor_sub(t0[:, :wd], Tin[:, k, o : o + wd], Mb)
                    nc.vector.tensor_mul(t0[:, :wd], t0[:, :wd], Rb)
                    nc.scalar.activation(
                        Tout[:, k, o : o + wd],
                        t0[:, :wd],
                        act_func,
                        bias=b_ap[:, k : k + 1],
                        scale=g_ap[:, k : k + 1],
                    )

        add = mybir.AluOpType.add

        for s in range(S):
            X = apool.tile([128, KD, C], bf16, tag="X", bufs=2, name="X")
            nc.sync.dma_start(out=X, in_=xg[s])
            v2 = vpool.tile([128, 6, KH], f32, tag="v2", name="v2")
            nc.sync.dma_start(out=v2, in_=vec2[s])
            v1 = vpool.tile([128, 2, KHH], f32, tag="v1", name="v1")
            nc.sync.dma_start(out=v1, in_=vec1[s])

            h0 = apool.tile([128, KH, C], bf16, tag="h0", bufs=1, name="h0")

            def epi1(m, o, wd, ps):
                nc.scalar.activation(
                    h0[:, m, o : o + wd], ps, AF.Relu, bias=v2[:, 0, m : m + 1]
                )

            mm_layer(s, w1, KD, KH, X, epi1, "w1")

            a1 = apool.tile([128, KH, C], bf16, tag="a1", bufs=1, name="a1")
            layernorm(h0, KH, v2[:, 1], v2[:, 2], AF.Relu, a1)

            h1 = apool.tile([128, KH, C], bf16, tag="h1", bufs=1, name="h1")

            def epi2(m, o, wd, ps):
                nc.vector.scalar_tensor_tensor(
                    out=h1[:, m, o : o + wd],
                    in0=ps,
                    scalar=v2[:, 3, m : m + 1],
                    in1=h0[:, m, o : o + wd],
                    op0=add,
                    op1=add,
                )

            mm_layer(s, w2, KH, KH, a1, epi2, "w2")

            a2 = apool.tile([128, KH, C], bf16, tag="a1", bufs=1, name="a2")
            layernorm(h1, KH, v2[:, 4], v2[:, 5], AF.Silu, a2)

            h2 = apool.tile([128, KHH, C], bf16, tag="h2", bufs=1, name="h2")

            def epi3(m, o, wd, ps):
                nc.scalar.activation(
                    h2[:, m, o : o + wd], ps, AF.Identity, bias=v1[:, 0, m : m + 1]
                )

            mm_layer(s, w3, KH, KHH, a2, epi3, "w3")

            y = apool.tile([128, KO, C], bf16, tag="y", bufs=1, name="y")

            def epi4(m, o, wd, ps):
                nc.scalar.activation(
                    y[:, m, o : o + wd], ps, AF.Identity, bias=v1[:, 1, m : m + 1]
                )

            mm_layer(s, w4, KHH, KO, h2, epi4, "w4")
            nc.sync.dma_start(out=yo[s], in_=y)

    nc.compile()
    _BUILD_CACHE[key] = nc
    return nc


# ----------------------------------------------------------------------------
# Host-side input packing
# ----------------------------------------------------------------------------

def _prep_expert_weights(inputs):
    """Per-expert weight tensors pre-tiled for the device DMA layout."""
    ew_in = np.asarray(inputs["ew_in"])
    ew_h1 = np.asarray(inputs["ew_h1"])
    ew_h2 = np.asarray(inputs["ew_h2"])
    ew_out = np.asarray(inputs["ew_out"])

    def tile_w(w, Kt, Mt):
        # [K, M] -> [Mt, 128(p=k%128), Kt, 128(j)] with w[k,m]=out[m//128, k%128, k//128, m%128]
        return np.ascontiguousarray(
            w.reshape(Kt, 128, Mt, 128).transpose(2, 1, 0, 3).astype(BF16)
        )

    per_e = []
    for e in range(E):
        per_e.append(
            dict(
                w1=tile_w(ew_in[e], KD, KH),
                w2=tile_w(ew_h1[e], KH, KH),
                w3=tile_w(ew_h2[e], KH, KHH),
                w4=tile_w(ew_out[e], KHH, KO),
            )
        )
    return per_e


def _prep_expert_vecs(inputs):
    names2 = ["eb_in", "ln1_g", "ln1_b", "eb_h1", "ln2_g", "ln2_b"]
    names1 = ["eb_h2", "eb_out"]
    per_e = []
    for e in range(E):
        v2 = np.stack(
            [np.asarray(inputs[n])[e].reshape(KH, 128).T for n in names2], axis=1
        ).astype(np.float32)  # [128, 6, KH]
        v1 = np.stack(
            [np.asarray(inputs[n])[e].reshape(KHH, 128).T for n in names1], axis=1
        ).astype(np.float32)  # [128, 2, KHH]
        per_e.append((np.ascontiguousarray(v2), np.ascontiguousarray(v1)))
    return per_e


# ----------------------------------------------------------------------------
# Main entry
# ----------------------------------------------------------------------------

def kernel(**inputs):
    global last_result
    from concourse.bass_utils import run_bass_kernel_spmd

    probs, topi, topp, loss = _gating(inputs)
    S, C, per_core = _plan(topi)

    # combine weight w[b, e]
    w_be = np.zeros((B, E), np.float32)
    np.put_along_axis(w_be, topi.astype(np.int64), topp, axis=1)

    nc = _build(S, C)

    x = np.asarray(inputs["x"], dtype=np.float32)
    xT_bf = np.ascontiguousarray(x.T.astype(BF16))  # [D, B] feature-major
    wts = _prep_expert_weights(inputs)
    vecs = _prep_expert_vecs(inputs)

    in_maps = []
    for c in range(N_CORES):
        xg_np = np.zeros((S, 128, KD, C), BF16)
        w1_np = np.empty((S, KH, 128, KD, 128), BF16)
        w2_np = np.empty((S, KH, 128, KH, 128), BF16)
        w3_np = np.empty((S, KHH, 128, KH, 128), BF16)
        w4_np = np.empty((S, KO, 128, KHH, 128), BF16)
        v2_np = np.empty((S, 128, 6, KH), np.float32)
        v1_np = np.empty((S, 128, 2, KHH), np.float32)
        for s, (e, idx) in enumerate(per_core[c]):
            n = len(idx)
            if n:
                # [D, n] -> [KD, 128, n] -> [128, KD, n]
                xg_np[s, :, :, :n] = (
                    xT_bf[:, idx].reshape(KD, 128, n).transpose(1, 0, 2)
                )
            w1_np[s] = wts[e]["w1"]
            w2_np[s] = wts[e]["w2"]
            w3_np[s] = wts[e]["w3"]
            w4_np[s] = wts[e]["w4"]
            v2_np[s], v1_np[s] = vecs[e]
        in_maps.append(
            dict(
                xg=xg_np, w1=w1_np, w2=w2_np, w3=w3_np, w4=w4_np,
                vec2=v2_np, vec1=v1_np,
            )
        )

    trace = os.environ.get("MOE_TRACE") == "1"
    res = run_bass_kernel_spmd(
        nc, in_maps, list(range(N_CORES)), trace=trace
    )
    last_result = res
    global _stash
    _stash = (nc, in_maps)

    final = np.zeros((B, O), np.float32)
    for c in range(N_CORES):  # noqa: duplicate loop var name fine
        yo_np = res.results[c]["yo"]  # [S, 128, KO, C] bf16
        for s, (e, idx) in enumerate(per_core[c]):
            n = len(idx)
            if not n:
                continue
            yf = (
                yo_np[s].transpose(1, 0, 2).reshape(O, C)[:, :n].astype(np.float32)
            )  # [O, n]
            final[idx] += yf.T * w_be[idx, e][:, None]

    return final, loss, probs


def bench_exec(n_inner=8, reps=3):
    """Estimate device execution time of the compiled SPMD kernel.

    Re-runs the exact PJRT executable with inputs pre-staged on device,
    chaining `n_inner` sequential kernel executions inside one jitted call
    (output buffers of call i feed call i+1, preventing CSE), so per-call
    wall time  ~=  dispatch_overhead/n_inner + T_kernel.
    Returns (per_call_ns_at_n_inner, per_call_ns_at_1).
    """
    import time

    import jax
    from jax.sharding import Mesh, NamedSharding, PartitionSpec
    from concourse import bass2jax
    from concourse.bass2jax import _bass_exec_p
    from concourse import mybir

    try:
        from jax.experimental.shard_map import shard_map
    except ImportError:
        from jax.shard_map import shard_map  # newer jax

    assert _stash is not None, "call kernel() first"
    nc, in_maps = _stash
    bass2jax.install_neuronx_cc_hook()

    part_name = nc.partition_id_tensor.name if nc.partition_id_tensor else None
    in_names, out_names, out_avals, zero_outs = [], [], [], []
    for alloc in nc.m.functions[0].allocations:
        if not isinstance(alloc, mybir.MemoryLocationSet):
            continue
        name = alloc.memorylocations[0].name
        if alloc.kind == "ExternalInput":
            if name != part_name:
                in_names.append(name)
        elif alloc.kind == "ExternalOutput":
            out_names.append(name)
            shape = tuple(alloc.tensor_shape)
            dtype = mybir.dt.np(alloc.dtype)
            out_avals.append(jax.core.ShapedArray(shape, dtype))
            zero_outs.append(np.zeros(shape, dtype))
    n_params = len(in_names)
    all_in_names = list(in_names + out_names)
    if part_name is not None:
        all_in_names.append(part_name)
    all_in_names = tuple(all_in_names)

    def _body(*args):
        operands = list(args)
        if part_name is not None:
            operands.append(bass2jax.partition_id_tensor())
        return tuple(
            _bass_exec_p.bind(
                *operands,
                out_avals=tuple(out_avals),
                in_names=all_in_names,
                out_names=tuple(out_names),
                lowering_input_output_aliases=(),
                sim_require_finite=True,
                sim_require_nnan=True,
                nc=nc,
            )
        )

    devices = jax.devices()[:N_CORES]
    mesh = Mesh(np.asarray(devices), ("core",))
    spec = NamedSharding(mesh, PartitionSpec("core"))
    concat_in = [
        np.concatenate([np.asarray(in_maps[c][k]) for c in range(N_CORES)], axis=0)
        for k in in_names
    ]
    concat_zeros = [
        np.zeros((N_CORES * z.shape[0], *z.shape[1:]), z.dtype) for z in zero_outs
    ]
    args_dev = [jax.device_put(a, spec) for a in concat_in + concat_zeros]
    jax.block_until_ready(args_dev)

    f = jax.jit(
        shard_map(
            _body,
            mesh=mesh,
            in_specs=(PartitionSpec("core"),) * (n_params + len(out_names)),
            out_specs=(PartitionSpec("core"),) * len(out_names),
            check_rep=False,
        ),
        keep_unused=True,
    )
    jax.block_until_ready(f(*args_dev))  # compile + warm

    def timed(n):
        best = float("inf")
        for _ in range(reps):
            t0 = time.perf_counter()
            out = None
            for _ in range(n):
                out = f(*args_dev)
            jax.block_until_ready(out)
            best = min(best, time.perf_counter() - t0)
        return best / n * 1e9

    t1 = timed(1)
    tn = timed(n_inner)
    return tn, t1
